# revision 21
# baseline (speedup 1.0000x reference)
"""Chamfer-distance (CDLoss) kernel for 8x TRN2 NeuronCores — v2.

Same two-phase certificate design as the baseline, plus:

* Variable per-block windows: each (direction, block) has a table width
  in {192..640} sized offline so only ~50 queries/direction need repair.
  Blocks are sorted by width and packed 4 to a PSUM tile; every PSUM
  tile is a constant [128, 4, 512] fp32 (the proven allocation pattern),
  with narrower windows written to [:, t, 0:W] and reduced via a strided
  AP.  Blocks grouped together are padded to the group max width
  (re-centred), which only improves their certificate.  640-wide blocks
  issue a 512-wide main matmul plus a 128-wide extra; the host
  min-merges the two output slots.

* Phase 2 fuses both directions into one matmul per candidate chunk by
  stacking them in the contraction dim with zero blocks (lhsT [14, 128]:
  rows 0-6 dir-0 query forms in columns 0-63, rows 7-13 dir-1 forms in
  columns 64-127; rhs rows 0-6/7-13 the two candidate clouds).  The zero
  blocks cancel cross terms exactly, so 16 matmuls and 4 reduces replace
  the baseline's 32 and 8.

Distances via the K=7 fp16 Gram matmul:
    d[n,m] = |x_n|^2 + |y_m|^2 - 2 x_n.y_m
    lhsT rows: [nhi_x, nlo_x, 1, 1, -2x0, -2x1, -2x2]
    rhs  rows: [1, 1, nhi_y, nlo_y, y0, y1, y2]
"""

import numpy as np

try:
    import concourse.bass as bass  # noqa: F401
except ImportError:  # harness environments without concourse on sys.path
    import sys

    sys.path.insert(0, "/opt/trn_rl_repo")

import concourse.bass as bass
import concourse.tile as tile
from concourse import mybir
from concourse.bass_utils import run_bass_kernel_spmd

B, N, M = 8, 8192, 8192
K = 7  # Gram-expansion contraction dim
NB = N // 128  # query blocks per batch
CAP = 64  # phase-2 repair queries per direction per round
MT = M // 512  # phase-2 candidate tiles
CERT_SLACK = 3e-4  # absolute fp16 noise absorbed into the certificate test
CERT_REL = 1.004  # relative fp16 noise factor on window minima
N_CORES = 8

# Per-(direction, block) window widths, sized offline from the z-sorted
# point statistics of N(0,1)^3 clouds so that <=~60 queries per direction
# miss their certificate (those are repaired exactly in phase 2).  Purely
# a performance table: any input stays correct via the repair path.
WT = [
    [192, 256, 320, 384, 320, 320, 320, 384, 320, 384, 448, 512, 448, 448,
     448, 448, 640, 512, 640, 640, 512, 448, 448, 448, 448, 512, 512, 512,
     512, 640, 640, 640, 640, 640, 640, 512, 512, 512, 512, 512, 512, 512,
     512, 640, 512, 640, 640, 640, 512, 512, 640, 640, 448, 384, 384, 384,
     320, 320, 320, 320, 320, 256, 192, 192],
    [192, 256, 256, 256, 320, 320, 320, 320, 384, 320, 448, 448, 448, 512,
     448, 448, 448, 640, 448, 640, 384, 448, 448, 512, 512, 512, 512, 640,
     640, 640, 640, 512, 640, 512, 512, 512, 512, 512, 512, 512, 448, 512,
     512, 512, 640, 640, 640, 512, 448, 448, 448, 512, 448, 512, 448, 384,
     384, 320, 320, 320, 320, 320, 256, 192],
]


def _forms(p):
    """fp16 lhsT/rhs Gram forms for one sorted cloud p [n, 3] fp32."""
    q = p.astype(np.float16)
    qf = q.astype(np.float32)
    nrm = (qf * qf).sum(-1)
    nh = nrm.astype(np.float16)
    nl = (nrm - nh.astype(np.float32)).astype(np.float16)
    one = np.ones_like(nh)
    lhsT = np.stack([nh, nl, one, one, -2 * q[:, 0], -2 * q[:, 1], -2 * q[:, 2]])
    rhs = np.stack([one, one, nh, nl, q[:, 0], q[:, 1], q[:, 2]])
    return lhsT, rhs


def _window(blk, W):
    return min(max(128 * blk + 64 - W // 2, 0), M - W)


def _plan():
    """Static phase-1 schedule.

    Returns (groups, cover, nslots).  groups: list of (d, Wg, items, col)
    with items = [(blk, start)], all windows in a group Wg wide, written
    to a [128, 4, 512] PSUM tile at [:, t, 0:Wg].  cover[(d, blk)] =
    (lo, hi) candidate-rank coverage for the certificate.  col indexes
    the packed mins output [128, nslots].
    """
    mains = {0: [], 1: []}  # (w_main, blk, start or None)
    extras = {0: [], 1: []}
    cover = {}
    for d in range(2):
        for blk, W in enumerate(WT[d]):
            if W <= 512:
                mains[d].append((W, blk, None))
            else:
                c = _window(blk, W)
                mains[d].append((512, blk, c))
                extras[d].append((blk, c + 512))
                cover[(d, blk)] = (c, c + W)
    groups = []
    col = 0
    for d in range(2):
        ms = sorted(mains[d])
        for i in range(0, len(ms), 4):
            chunk = ms[i : i + 4]
            Wg = max(w for w, _, _ in chunk)
            items = []
            for w, blk, start in chunk:
                if start is None:  # pad to group width, re-centre
                    start = _window(blk, Wg)
                    cover[(d, blk)] = (start, start + Wg)
                items.append((blk, start))
            groups.append((d, Wg, items, col))
            col += len(items)
    for d in range(2):
        ex = extras[d]
        for i in range(0, len(ex), 4):
            chunk = ex[i : i + 4]
            groups.append((d, 128, chunk, col))
            col += len(chunk)
    return groups, cover, col


def _elide_redundant_waits(nc):
    """Drop transitively-redundant sem waits so every instruction has <=1.

    The walrus build in this image rejects instructions carrying more than
    one sync wait ("Too many sync wait commands").  Tile emits per-proc
    minimal waits but not transitively-minimal ones.  We compute, per
    instruction in committed order, the vector-clock of sem values each
    engine has provably observed - inheriting the updater's clock when
    waiting on a semaphore - and drop any wait implied by another wait on
    the same instruction or already observed by the engine.  Remaining
    multi-waits are hoisted onto same-engine NoOps.
    """
    import copy as _copy

    blocks = nc.m.functions[0].blocks
    insts = [i for blk in blocks for i in blk.instructions]
    loc = {}
    for blk in blocks:
        for i in blk.instructions:
            loc[i.name] = blk
    obs = {}
    cum = {}
    snaps = {}

    def snap_at(sem, val):
        for cv, snap in snaps.get(sem, ()):
            if cv >= val:
                return snap
        return None

    for inst in insts:
        si = inst.sync_info
        eng = inst.engine
        o = obs.setdefault(eng, {})
        if si and si.on_wait:
            waits = list(si.on_wait)
            kept = list(waits)
            changed = True
            while changed and len(kept) > 1:
                changed = False
                for k, w in enumerate(kept):
                    others = kept[:k] + kept[k + 1 :]
                    imp = o.get(w.ant_name, 0) >= w.wait_value
                    for w2 in others:
                        if imp:
                            break
                        if w2.ant_name == w.ant_name and w2.wait_value >= w.wait_value:
                            imp = True
                            break
                        snap = snap_at(w2.ant_name, w2.wait_value)
                        if snap is not None and snap.get(w.ant_name, 0) >= w.wait_value:
                            imp = True
                    if imp:
                        kept.pop(k)
                        changed = True
                        break
            if len(kept) > 1:
                blk = loc[inst.name]
                pos = next(
                    k for k, i2 in enumerate(blk.instructions) if i2.name == inst.name
                )
                for j, w in enumerate(kept[:-1]):
                    nop = mybir.InstNoOp(name=f"{inst.name}-hw{j}", ins=[], outs=[])
                    nop.engine = eng
                    nsi = _copy.deepcopy(si)
                    nsi.on_wait[:] = [w]
                    if nsi.on_update:
                        nsi.on_update[:] = []
                    nop.sync_info = nsi
                    blk.instructions.insert(pos + j, nop)
                kept = kept[-1:]
            si.on_wait[:] = kept
            for w in waits:
                if o.get(w.ant_name, 0) < w.wait_value:
                    o[w.ant_name] = w.wait_value
                snap = snap_at(w.ant_name, w.wait_value)
                if snap is not None:
                    for s, v in snap.items():
                        if o.get(s, 0) < v:
                            o[s] = v
        if si and si.on_update:
            for u in si.on_update:
                name = u.ant_name
                inc = getattr(u, "value", None) or getattr(u, "update_value", None)
                if inc is None:
                    inc = 16 if name.startswith("DMA") else 1
                cum[name] = cum.get(name, 0) + inc
                snaps.setdefault(name, []).append((cum[name], dict(o)))


def _build_phase1(plan, nslots):
    f16, f32 = mybir.dt.float16, mybir.dt.float32
    X, MIN = mybir.AxisListType.X, mybir.AluOpType.min

    nc = bass.Bass()
    # pts[:, 0]=lhsT(x), 1=rhs(y), 2=lhsT(y), 3=rhs(x); all z-sorted
    pts = nc.declare_dram_parameter("pts", [K, 4, N], f16, isOutput=False)
    mins = nc.declare_dram_parameter("mins", [128, nslots], f32, isOutput=True)

    with tile.TileContext(nc) as tc:
        with (
            tc.tile_pool(name="singles", bufs=1) as singles,
            tc.tile_pool(name="psum", bufs=2, space="PSUM") as psum,
        ):
            P = singles.tile([K, 4, N], f16)
            # groups are emitted narrowest-W first, and narrow blocks sit at
            # the rank extremes: load each direction's planes edges-first in
            # quarter chunks so the first matmuls start as early as possible
            Q4 = N // 4
            for cp in (0, 2):
                for h in (0, 3, 1, 2):
                    nc.sync.dma_start(
                        out=P[:, cp : cp + 2, h * Q4 : (h + 1) * Q4],
                        in_=pts[:, cp : cp + 2, h * Q4 : (h + 1) * Q4],
                    )
            mt = singles.tile([128, nslots], f32)

            for d, Wg, items, col in plan:
                nb = len(items)
                pt = psum.tile([128, 4, 512], f32, tag="pt")
                for t, (blk, c) in enumerate(items):
                    nc.tensor.matmul(
                        pt[:, t, 0:Wg],
                        P[:, 2 * d, 128 * blk : 128 * blk + 128],
                        P[:, 2 * d + 1, c : c + Wg],
                        start=True,
                        stop=True,
                    )
                if nb == 4 and Wg >= 448:
                    # wide groups: split the reduce so the first half runs
                    # under the group's last matmuls and the PSUM bank frees
                    # earlier (shrinks the PE stall at the group boundary)
                    nc.vector.tensor_reduce(
                        mt[:, col : col + 2], pt[:, 0:2, 0:Wg], axis=X, op=MIN
                    )
                    nc.vector.tensor_reduce(
                        mt[:, col + 2 : col + 4], pt[:, 2:4, 0:Wg], axis=X, op=MIN
                    )
                else:
                    nc.vector.tensor_reduce(
                        mt[:, col : col + nb], pt[:, 0:nb, 0:Wg], axis=X, op=MIN
                    )
            nc.sync.dma_start(out=mins[:, :], in_=mt[:, :])

    _elide_redundant_waits(nc)
    return nc


def _build_phase2():
    """Both directions fused into one matmul per candidate chunk.

    lhsT [2K, 128]: rows 0-6 carry dir-0 query forms in columns 0-63
    (zeros elsewhere), rows 7-13 carry dir-1 forms in columns 64-127.
    rhs [2K, 512]: rows 0-6 = dir-0 candidates, rows 7-13 = dir-1.  The
    zero blocks cancel the cross terms exactly, so partition rows 0-63
    see dir-0 distances and rows 64-127 dir-1, in a single matmul.
    """
    f16, f32 = mybir.dt.float16, mybir.dt.float32
    MIN = mybir.AluOpType.min

    nc = bass.Bass()
    # chunk grouping: a small final group shortens the critical-path tail
    # (the last reduce runs right after the last matmul)
    GS = [4, 4, 4, 3, 1]
    q2 = nc.declare_dram_parameter("q2", [2 * K, 128], f16, isOutput=False)
    cand = nc.declare_dram_parameter("cand", [2 * K, M], f16, isOutput=False)
    mins2 = nc.declare_dram_parameter("mins2", [128, len(GS)], f32, isOutput=True)

    with tile.TileContext(nc) as tc:
        with (
            tc.tile_pool(name="singles", bufs=1) as singles,
            tc.tile_pool(name="psum", bufs=2, space="PSUM") as psum,
        ):
            Q = singles.tile([2 * K, 128], f16)
            C = singles.tile([2 * K, M], f16)
            Q4 = M // 4
            nc.sync.dma_start(out=Q, in_=q2[:, :])
            # chunks in scan order so the matmul chain starts on chunk 0
            for h in (0, 1, 2, 3):
                nc.sync.dma_start(
                    out=C[:, h * Q4 : (h + 1) * Q4],
                    in_=cand[:, h * Q4 : (h + 1) * Q4],
                )
            m2 = singles.tile([128, len(GS)], f32)

            j = 0
            for g, ng in enumerate(GS):
                pt = psum.tile([128, 4, 512], f32, tag="pt")
                for t in range(ng):
                    nc.tensor.matmul(
                        pt[:, t, :],
                        Q,
                        C[:, 512 * j : 512 * j + 512],
                        start=True,
                        stop=True,
                    )
                    j += 1
                nc.vector.tensor_reduce(
                    m2[:, g : g + 1], pt[:, 0:ng, :], axis=mybir.AxisListType.XY, op=MIN
                )
            nc.sync.dma_start(out=mins2[:, :], in_=m2[:, :])

    _elide_redundant_waits(nc)
    return nc


def _install_ntff_hook():
    """Provide antenv.axon_hooks (absent in this image) so trace=True works."""
    import contextlib
    import ctypes
    import sys
    import types

    if "antenv.axon_hooks" in sys.modules:
        return
    hook = None
    try:
        lib = ctypes.CDLL("/opt/axon/libaxon_pjrt.so")
        if hasattr(lib, "axon_start_nrt_profile"):
            lib.axon_start_nrt_profile.argtypes = [
                ctypes.POINTER(ctypes.c_int64),
                ctypes.c_size_t,
            ]
            lib.axon_start_nrt_profile.restype = ctypes.c_int64
            lib.axon_stop_nrt_profile.argtypes = [ctypes.c_char_p]
            lib.axon_stop_nrt_profile.restype = ctypes.c_int64

            @contextlib.contextmanager
            def _hook(output_dir, device_ids):
                import jax

                jax.devices()
                if device_ids:
                    ids = (ctypes.c_int64 * len(device_ids))(*device_ids)
                    rc = lib.axon_start_nrt_profile(ids, len(device_ids))
                else:
                    rc = lib.axon_start_nrt_profile(None, 0)
                if rc != 0:
                    raise RuntimeError(f"axon_start_nrt_profile rc={rc}")
                try:
                    yield
                finally:
                    n = lib.axon_stop_nrt_profile(str(output_dir).encode())
                    print(f"profile: {n} file(s) written to {output_dir}")

            hook = _hook
    except OSError:
        pass

    mod = types.ModuleType("antenv.axon_hooks")
    mod.get_axon_ntff_profile_hook = lambda: hook
    mod.set_axon_ntff_profile_hook = lambda h: None
    sys.modules["antenv.axon_hooks"] = mod

    from concourse import bass_utils

    bass_utils.upload_artifacts = lambda tmpdir: f"local://{tmpdir}"


def _cert(zq, zc, cover, d):
    """Exactness bound per query rank: margin^2 to the nearest live edge."""
    cert = np.empty(len(zq), np.float64)
    for blk in range(len(zq) // 128):
        lo_i, hi_i = cover[(d, blk)]
        xs = slice(128 * blk, 128 * blk + 128)
        lo = zq[xs] - zc[lo_i] if lo_i > 0 else np.full(128, np.inf)
        hi = zc[hi_i - 1] - zq[xs] if hi_i < len(zc) else np.full(128, np.inf)
        m = np.minimum(lo, hi)
        cert[xs] = np.where(m > 0, m * m, 0.0)
    return cert


def _prep(pcs1, pcs2):
    batches = []
    in_maps1 = []
    for b in range(B):
        i1 = np.argsort(pcs1[b, :, 2], kind="stable")
        i2 = np.argsort(pcs2[b, :, 2], kind="stable")
        x = pcs1[b][i1]
        y = pcs2[b][i2]
        l1, r1 = _forms(x)
        l2, r2 = _forms(y)
        pts = np.stack([l1, r2, l2, r1], axis=1)
        in_maps1.append({"pts": np.ascontiguousarray(pts, dtype=np.float16)})
        batches.append(
            (x[:, 2].astype(np.float64), y[:, 2].astype(np.float64), l1, r1, l2, r2)
        )
    return batches, in_maps1


def kernel(pcs1, pcs2, _trace=False):
    pcs1 = np.asarray(pcs1, dtype=np.float32)
    pcs2 = np.asarray(pcs2, dtype=np.float32)
    if _trace:
        _install_ntff_hook()

    plan, cover, nslots = _plan()
    batches, in_maps1 = _prep(pcs1, pcs2)

    cores = list(range(N_CORES))
    res1 = run_bass_kernel_spmd(
        _build_phase1(plan, nslots), in_maps1, cores, trace=_trace
    )
    t1 = res1.exec_time_ns

    # unpack group-packed mins to (d, blk) rank order, min-merging extras
    fails_all = []
    vals_all = []
    nrounds = 1
    for b in range(B):
        z1, z2, l1, r1, l2, r2 = batches[b]
        mtp = np.asarray(res1.results[b]["mins"], dtype=np.float64)
        wmins = np.full((2, N), np.inf)
        for d, Wg, items, col in plan:
            for t, (blk, c) in enumerate(items):
                xs = slice(128 * blk, 128 * blk + 128)
                wmins[d, xs] = np.minimum(wmins[d, xs], mtp[:, col + t])
        dir_fails = []
        dir_vals = []
        for d, (zq, zc) in enumerate(((z1, z2), (z2, z1))):
            wm = wmins[d]
            fails = np.where(wm * CERT_REL > _cert(zq, zc, cover, d) - CERT_SLACK)[0]
            nrounds = max(nrounds, -(-len(fails) // CAP))
            dir_fails.append(fails)
            dir_vals.append(wm.copy())
        fails_all.append(dir_fails)
        vals_all.append(dir_vals)

    # phase-2 exact repair; multiple rounds if >CAP queries fail anywhere
    nc2 = _build_phase2()
    t2 = 0
    for rnd in range(nrounds):
        in_maps2 = []
        for b in range(B):
            _, _, l1, r1, l2, r2 = batches[b]
            qsel = np.zeros((2 * K, 128), np.float16)
            for d, lq in enumerate((l1, l2)):
                fl = fails_all[b][d][rnd * CAP : (rnd + 1) * CAP]
                if len(fl):
                    qsel[K * d : K * d + K, 64 * d : 64 * d + len(fl)] = lq[:, fl]
            in_maps2.append(
                {
                    "q2": qsel,
                    "cand": np.ascontiguousarray(
                        np.concatenate([r2, r1], axis=0), np.float16
                    ),
                }
            )
        res2 = run_bass_kernel_spmd(nc2, in_maps2, cores, trace=_trace)
        if _trace and res2.exec_time_ns is not None:
            t2 += res2.exec_time_ns
        for b in range(B):
            m2 = np.asarray(res2.results[b]["mins2"], dtype=np.float64).min(-1)
            for d in range(2):
                fl = fails_all[b][d][rnd * CAP : (rnd + 1) * CAP]
                vals_all[b][d][fl] = m2[64 * d : 64 * d + len(fl)]

    if _trace and t1 is not None:
        print(f"HW exec time: {t1 + t2} ns (phase1 {t1} + phase2 {t2} x{nrounds})")

    total = np.float64(0.0)
    for b in range(B):
        for d in range(2):
            total += np.maximum(vals_all[b][d], 0.0).sum()
    return np.float32(total / (B * N))


# revision 22
# speedup vs baseline: 1.1421x; 1.1421x over previous
"""Chamfer-distance (CDLoss) kernel for 8x TRN2 NeuronCores — v2.

Same two-phase certificate design as the baseline, plus:

* Variable per-block windows: each (direction, block) has a table width
  in {192..640} sized offline so only ~50 queries/direction need repair.
  Blocks are sorted by width and packed 4 to a PSUM tile; every PSUM
  tile is a constant [128, 4, 512] fp32 (the proven allocation pattern),
  with narrower windows written to [:, t, 0:W] and reduced via a strided
  AP.  Blocks grouped together are padded to the group max width
  (re-centred), which only improves their certificate.  640-wide blocks
  issue a 512-wide main matmul plus a 128-wide extra; the host
  min-merges the two output slots.

* Phase 2 fuses both directions into one matmul per candidate chunk by
  stacking them in the contraction dim with zero blocks (lhsT [14, 128]:
  rows 0-6 dir-0 query forms in columns 0-63, rows 7-13 dir-1 forms in
  columns 64-127; rhs rows 0-6/7-13 the two candidate clouds).  The zero
  blocks cancel cross terms exactly, so 16 matmuls and 4 reduces replace
  the baseline's 32 and 8.

Distances via the K=7 fp16 Gram matmul:
    d[n,m] = |x_n|^2 + |y_m|^2 - 2 x_n.y_m
    lhsT rows: [nhi_x, nlo_x, 1, 1, -2x0, -2x1, -2x2]
    rhs  rows: [1, 1, nhi_y, nlo_y, y0, y1, y2]
"""

import numpy as np

try:
    import concourse.bass as bass  # noqa: F401
except ImportError:  # harness environments without concourse on sys.path
    import sys

    sys.path.insert(0, "/opt/trn_rl_repo")

import concourse.bass as bass
import concourse.tile as tile
from concourse import mybir
from concourse.bass_utils import run_bass_kernel_spmd

B, N, M = 8, 8192, 8192
K = 7  # Gram-expansion contraction dim
NB = N // 128  # query blocks per batch
CAP = 64  # phase-2 repair queries per direction per round
MT = M // 512  # phase-2 candidate tiles
CERT_SLACK = 3e-4  # absolute fp16 noise absorbed into the certificate test
CERT_REL = 1.004  # relative fp16 noise factor on window minima
N_CORES = 8

# Per-(direction, block) window widths, sized offline from the z-sorted
# point statistics of N(0,1)^3 clouds so that <=~60 queries per direction
# miss their certificate (those are repaired exactly in phase 2).  Purely
# a performance table: any input stays correct via the repair path.
WT = [
    [192, 256, 320, 384, 320, 320, 320, 384, 320, 384, 448, 512, 448, 448,
     448, 448, 640, 512, 640, 640, 512, 448, 448, 448, 448, 512, 512, 512,
     512, 640, 640, 640, 640, 640, 640, 512, 512, 512, 512, 512, 512, 512,
     512, 640, 512, 640, 640, 640, 512, 512, 640, 640, 448, 384, 384, 384,
     320, 320, 320, 320, 320, 256, 192, 192],
    [192, 256, 256, 256, 320, 320, 320, 320, 384, 320, 448, 448, 448, 512,
     448, 448, 448, 640, 448, 640, 384, 448, 448, 512, 512, 512, 512, 640,
     640, 640, 640, 512, 640, 512, 512, 512, 512, 512, 512, 512, 448, 512,
     512, 512, 640, 640, 640, 512, 448, 448, 448, 512, 448, 512, 448, 384,
     384, 320, 320, 320, 320, 320, 256, 192],
]


def _forms(p):
    """fp16 lhsT/rhs Gram forms for one sorted cloud p [n, 3] fp32."""
    q = p.astype(np.float16)
    qf = q.astype(np.float32)
    nrm = (qf * qf).sum(-1)
    nh = nrm.astype(np.float16)
    nl = (nrm - nh.astype(np.float32)).astype(np.float16)
    one = np.ones_like(nh)
    lhsT = np.stack([nh, nl, one, one, -2 * q[:, 0], -2 * q[:, 1], -2 * q[:, 2]])
    rhs = np.stack([one, one, nh, nl, q[:, 0], q[:, 1], q[:, 2]])
    return lhsT, rhs


def _window(blk, W):
    return min(max(128 * blk + 64 - W // 2, 0), M - W)


def _plan():
    """Static phase-1 schedule.

    Returns (groups, cover, nslots).  groups: list of (d, Wg, items, col)
    with items = [(blk, start)], all windows in a group Wg wide, written
    to a [128, 4, 512] PSUM tile at [:, t, 0:Wg].  cover[(d, blk)] =
    (lo, hi) candidate-rank coverage for the certificate.  col indexes
    the packed mins output [128, nslots].
    """
    mains = {0: [], 1: []}  # (w_main, blk, start or None)
    extras = {0: [], 1: []}
    cover = {}
    for d in range(2):
        for blk, W in enumerate(WT[d]):
            if W <= 512:
                mains[d].append((W, blk, None))
            else:
                c = _window(blk, W)
                mains[d].append((512, blk, c))
                extras[d].append((blk, c + 512))
                cover[(d, blk)] = (c, c + W)
    groups = []
    col = 0
    for d in range(2):
        ms = sorted(mains[d])
        for i in range(0, len(ms), 4):
            chunk = ms[i : i + 4]
            Wg = max(w for w, _, _ in chunk)
            items = []
            for w, blk, start in chunk:
                if start is None:  # pad to group width, re-centre
                    start = _window(blk, Wg)
                    cover[(d, blk)] = (start, start + Wg)
                items.append((blk, start))
            groups.append((d, Wg, items, col))
            col += len(items)
    for d in range(2):
        ex = extras[d]
        for i in range(0, len(ex), 4):
            chunk = ex[i : i + 4]
            groups.append((d, 128, chunk, col))
            col += len(chunk)
    return groups, cover, col


def _elide_redundant_waits(nc):
    """Drop transitively-redundant sem waits so every instruction has <=1.

    The walrus build in this image rejects instructions carrying more than
    one sync wait ("Too many sync wait commands").  Tile emits per-proc
    minimal waits but not transitively-minimal ones.  We compute, per
    instruction in committed order, the vector-clock of sem values each
    engine has provably observed - inheriting the updater's clock when
    waiting on a semaphore - and drop any wait implied by another wait on
    the same instruction or already observed by the engine.  Remaining
    multi-waits are hoisted onto same-engine NoOps.
    """
    import copy as _copy

    blocks = nc.m.functions[0].blocks
    insts = [i for blk in blocks for i in blk.instructions]
    loc = {}
    for blk in blocks:
        for i in blk.instructions:
            loc[i.name] = blk
    obs = {}
    cum = {}
    snaps = {}

    def snap_at(sem, val):
        for cv, snap in snaps.get(sem, ()):
            if cv >= val:
                return snap
        return None

    for inst in insts:
        si = inst.sync_info
        eng = inst.engine
        o = obs.setdefault(eng, {})
        if si and si.on_wait:
            waits = list(si.on_wait)
            kept = list(waits)
            changed = True
            while changed and len(kept) > 1:
                changed = False
                for k, w in enumerate(kept):
                    others = kept[:k] + kept[k + 1 :]
                    imp = o.get(w.ant_name, 0) >= w.wait_value
                    for w2 in others:
                        if imp:
                            break
                        if w2.ant_name == w.ant_name and w2.wait_value >= w.wait_value:
                            imp = True
                            break
                        snap = snap_at(w2.ant_name, w2.wait_value)
                        if snap is not None and snap.get(w.ant_name, 0) >= w.wait_value:
                            imp = True
                    if imp:
                        kept.pop(k)
                        changed = True
                        break
            if len(kept) > 1:
                blk = loc[inst.name]
                pos = next(
                    k for k, i2 in enumerate(blk.instructions) if i2.name == inst.name
                )
                for j, w in enumerate(kept[:-1]):
                    nop = mybir.InstNoOp(name=f"{inst.name}-hw{j}", ins=[], outs=[])
                    nop.engine = eng
                    nsi = _copy.deepcopy(si)
                    nsi.on_wait[:] = [w]
                    if nsi.on_update:
                        nsi.on_update[:] = []
                    nop.sync_info = nsi
                    blk.instructions.insert(pos + j, nop)
                kept = kept[-1:]
            si.on_wait[:] = kept
            for w in waits:
                if o.get(w.ant_name, 0) < w.wait_value:
                    o[w.ant_name] = w.wait_value
                snap = snap_at(w.ant_name, w.wait_value)
                if snap is not None:
                    for s, v in snap.items():
                        if o.get(s, 0) < v:
                            o[s] = v
        if si and si.on_update:
            for u in si.on_update:
                name = u.ant_name
                inc = getattr(u, "value", None) or getattr(u, "update_value", None)
                if inc is None:
                    inc = 16 if name.startswith("DMA") else 1
                cum[name] = cum.get(name, 0) + inc
                snaps.setdefault(name, []).append((cum[name], dict(o)))


def _build_phase1(plan, nslots):
    f16, f32 = mybir.dt.float16, mybir.dt.float32
    X, MIN = mybir.AxisListType.X, mybir.AluOpType.min

    nc = bass.Bass()
    # pts[:, 0]=lhsT(x), 1=rhs(y), 2=lhsT(y), 3=rhs(x); all z-sorted
    pts = nc.declare_dram_parameter("pts", [K, 4, N], f16, isOutput=False)
    mins = nc.declare_dram_parameter("mins", [128, nslots], f32, isOutput=True)

    with tile.TileContext(nc) as tc:
        with (
            tc.tile_pool(name="singles", bufs=1) as singles,
            tc.tile_pool(name="psum", bufs=2, space="PSUM") as psum,
        ):
            P = singles.tile([K, 4, N], f16)
            # groups are emitted narrowest-W first, and narrow blocks sit at
            # the rank extremes: load each direction's planes edges-first in
            # quarter chunks so the first matmuls start as early as possible
            Q4 = N // 4
            for cp in (0, 2):
                for h in (0, 3, 1, 2):
                    nc.sync.dma_start(
                        out=P[:, cp : cp + 2, h * Q4 : (h + 1) * Q4],
                        in_=pts[:, cp : cp + 2, h * Q4 : (h + 1) * Q4],
                    )
            mt = singles.tile([128, nslots], f32)

            for d, Wg, items, col in plan:
                nb = len(items)
                pt = psum.tile([128, 4, 512], f32, tag="pt")
                for t, (blk, c) in enumerate(items):
                    nc.tensor.matmul(
                        pt[:, t, 0:Wg],
                        P[:, 2 * d, 128 * blk : 128 * blk + 128],
                        P[:, 2 * d + 1, c : c + Wg],
                        start=True,
                        stop=True,
                    )
                nc.vector.tensor_reduce(
                    mt[:, col : col + nb], pt[:, 0:nb, 0:Wg], axis=X, op=MIN
                )
            nc.sync.dma_start(out=mins[:, :], in_=mt[:, :])

    _elide_redundant_waits(nc)
    return nc


def _build_phase2():
    """Both directions fused into one matmul per candidate chunk.

    lhsT [2K, 128]: rows 0-6 carry dir-0 query forms in columns 0-63
    (zeros elsewhere), rows 7-13 carry dir-1 forms in columns 64-127.
    rhs [2K, 512]: rows 0-6 = dir-0 candidates, rows 7-13 = dir-1.  The
    zero blocks cancel the cross terms exactly, so partition rows 0-63
    see dir-0 distances and rows 64-127 dir-1, in a single matmul.
    """
    f16, f32 = mybir.dt.float16, mybir.dt.float32
    MIN = mybir.AluOpType.min

    nc = bass.Bass()
    # chunk grouping: a small final group shortens the critical-path tail
    # (the last reduce runs right after the last matmul)
    GS = [4, 4, 4, 3, 1]
    q2 = nc.declare_dram_parameter("q2", [2 * K, 128], f16, isOutput=False)
    cand = nc.declare_dram_parameter("cand", [2 * K, M], f16, isOutput=False)
    mins2 = nc.declare_dram_parameter("mins2", [128, len(GS)], f32, isOutput=True)

    with tile.TileContext(nc) as tc:
        with (
            tc.tile_pool(name="singles", bufs=1) as singles,
            tc.tile_pool(name="psum", bufs=2, space="PSUM") as psum,
        ):
            Q = singles.tile([2 * K, 128], f16)
            C = singles.tile([2 * K, M], f16)
            Q4 = M // 4
            nc.sync.dma_start(out=Q, in_=q2[:, :])
            # chunks in scan order so the matmul chain starts on chunk 0
            for h in (0, 1, 2, 3):
                nc.sync.dma_start(
                    out=C[:, h * Q4 : (h + 1) * Q4],
                    in_=cand[:, h * Q4 : (h + 1) * Q4],
                )
            m2 = singles.tile([128, len(GS)], f32)

            j = 0
            for g, ng in enumerate(GS):
                pt = psum.tile([128, 4, 512], f32, tag="pt")
                for t in range(ng):
                    nc.tensor.matmul(
                        pt[:, t, :],
                        Q,
                        C[:, 512 * j : 512 * j + 512],
                        start=True,
                        stop=True,
                    )
                    j += 1
                nc.vector.tensor_reduce(
                    m2[:, g : g + 1], pt[:, 0:ng, :], axis=mybir.AxisListType.XY, op=MIN
                )
            nc.sync.dma_start(out=mins2[:, :], in_=m2[:, :])

    _elide_redundant_waits(nc)
    return nc


def _install_ntff_hook():
    """Provide antenv.axon_hooks (absent in this image) so trace=True works."""
    import contextlib
    import ctypes
    import sys
    import types

    if "antenv.axon_hooks" in sys.modules:
        return
    hook = None
    try:
        lib = ctypes.CDLL("/opt/axon/libaxon_pjrt.so")
        if hasattr(lib, "axon_start_nrt_profile"):
            lib.axon_start_nrt_profile.argtypes = [
                ctypes.POINTER(ctypes.c_int64),
                ctypes.c_size_t,
            ]
            lib.axon_start_nrt_profile.restype = ctypes.c_int64
            lib.axon_stop_nrt_profile.argtypes = [ctypes.c_char_p]
            lib.axon_stop_nrt_profile.restype = ctypes.c_int64

            @contextlib.contextmanager
            def _hook(output_dir, device_ids):
                import jax

                jax.devices()
                if device_ids:
                    ids = (ctypes.c_int64 * len(device_ids))(*device_ids)
                    rc = lib.axon_start_nrt_profile(ids, len(device_ids))
                else:
                    rc = lib.axon_start_nrt_profile(None, 0)
                if rc != 0:
                    raise RuntimeError(f"axon_start_nrt_profile rc={rc}")
                try:
                    yield
                finally:
                    n = lib.axon_stop_nrt_profile(str(output_dir).encode())
                    print(f"profile: {n} file(s) written to {output_dir}")

            hook = _hook
    except OSError:
        pass

    mod = types.ModuleType("antenv.axon_hooks")
    mod.get_axon_ntff_profile_hook = lambda: hook
    mod.set_axon_ntff_profile_hook = lambda h: None
    sys.modules["antenv.axon_hooks"] = mod

    from concourse import bass_utils

    bass_utils.upload_artifacts = lambda tmpdir: f"local://{tmpdir}"


def _cert(zq, zc, cover, d):
    """Exactness bound per query rank: margin^2 to the nearest live edge."""
    cert = np.empty(len(zq), np.float64)
    for blk in range(len(zq) // 128):
        lo_i, hi_i = cover[(d, blk)]
        xs = slice(128 * blk, 128 * blk + 128)
        lo = zq[xs] - zc[lo_i] if lo_i > 0 else np.full(128, np.inf)
        hi = zc[hi_i - 1] - zq[xs] if hi_i < len(zc) else np.full(128, np.inf)
        m = np.minimum(lo, hi)
        cert[xs] = np.where(m > 0, m * m, 0.0)
    return cert


def _prep(pcs1, pcs2):
    batches = []
    in_maps1 = []
    for b in range(B):
        i1 = np.argsort(pcs1[b, :, 2], kind="stable")
        i2 = np.argsort(pcs2[b, :, 2], kind="stable")
        x = pcs1[b][i1]
        y = pcs2[b][i2]
        l1, r1 = _forms(x)
        l2, r2 = _forms(y)
        pts = np.stack([l1, r2, l2, r1], axis=1)
        in_maps1.append({"pts": np.ascontiguousarray(pts, dtype=np.float16)})
        batches.append(
            (x[:, 2].astype(np.float64), y[:, 2].astype(np.float64), l1, r1, l2, r2)
        )
    return batches, in_maps1


def kernel(pcs1, pcs2, _trace=False):
    pcs1 = np.asarray(pcs1, dtype=np.float32)
    pcs2 = np.asarray(pcs2, dtype=np.float32)
    if _trace:
        _install_ntff_hook()

    plan, cover, nslots = _plan()
    batches, in_maps1 = _prep(pcs1, pcs2)

    cores = list(range(N_CORES))
    res1 = run_bass_kernel_spmd(
        _build_phase1(plan, nslots), in_maps1, cores, trace=_trace
    )
    t1 = res1.exec_time_ns

    # unpack group-packed mins to (d, blk) rank order, min-merging extras
    fails_all = []
    vals_all = []
    nrounds = 1
    for b in range(B):
        z1, z2, l1, r1, l2, r2 = batches[b]
        mtp = np.asarray(res1.results[b]["mins"], dtype=np.float64)
        wmins = np.full((2, N), np.inf)
        for d, Wg, items, col in plan:
            for t, (blk, c) in enumerate(items):
                xs = slice(128 * blk, 128 * blk + 128)
                wmins[d, xs] = np.minimum(wmins[d, xs], mtp[:, col + t])
        dir_fails = []
        dir_vals = []
        for d, (zq, zc) in enumerate(((z1, z2), (z2, z1))):
            wm = wmins[d]
            fails = np.where(wm * CERT_REL > _cert(zq, zc, cover, d) - CERT_SLACK)[0]
            nrounds = max(nrounds, -(-len(fails) // CAP))
            dir_fails.append(fails)
            dir_vals.append(wm.copy())
        fails_all.append(dir_fails)
        vals_all.append(dir_vals)

    # phase-2 exact repair; multiple rounds if >CAP queries fail anywhere
    nc2 = _build_phase2()
    t2 = 0
    for rnd in range(nrounds):
        in_maps2 = []
        for b in range(B):
            _, _, l1, r1, l2, r2 = batches[b]
            qsel = np.zeros((2 * K, 128), np.float16)
            for d, lq in enumerate((l1, l2)):
                fl = fails_all[b][d][rnd * CAP : (rnd + 1) * CAP]
                if len(fl):
                    qsel[K * d : K * d + K, 64 * d : 64 * d + len(fl)] = lq[:, fl]
            in_maps2.append(
                {
                    "q2": qsel,
                    "cand": np.ascontiguousarray(
                        np.concatenate([r2, r1], axis=0), np.float16
                    ),
                }
            )
        res2 = run_bass_kernel_spmd(nc2, in_maps2, cores, trace=_trace)
        if _trace and res2.exec_time_ns is not None:
            t2 += res2.exec_time_ns
        for b in range(B):
            m2 = np.asarray(res2.results[b]["mins2"], dtype=np.float64).min(-1)
            for d in range(2):
                fl = fails_all[b][d][rnd * CAP : (rnd + 1) * CAP]
                vals_all[b][d][fl] = m2[64 * d : 64 * d + len(fl)]

    if _trace and t1 is not None:
        print(f"HW exec time: {t1 + t2} ns (phase1 {t1} + phase2 {t2} x{nrounds})")

    total = np.float64(0.0)
    for b in range(B):
        for d in range(2):
            total += np.maximum(vals_all[b][d], 0.0).sum()
    return np.float32(total / (B * N))
